# revision 3
# baseline (speedup 1.0000x reference)
"""VQ codebook reconstruction kernel for Trainium2 (8 NeuronCores, SPMD).

Reference computation (per pixel feature vector f in R^C):
    weights = (codebook @ f) / ||codebook_rows||^2      # [N]
    recon   = codebook.T @ weights                      # [C]

This collapses to a single fixed matrix applied per pixel:
    recon = M @ f,   M = codebook.T @ diag(1/||c_n||^2) @ codebook   # [C, C]

M is tiny ([256,256]) and is formed on the host in float64; the device
kernel applies M to all B*H*W = 131072 pixel vectors, sharded
data-parallel over (B, H) across 8 cores.

v3 structure (vs 65 us v2, 81 us v1):
  - feature and M in bf16 (host cast, ~2^-9 rel err), output fp16.
  - 16 chunks of 1024 columns; reads alternate the two HWDGE rings
    (sync/scalar) and are all issued up front (16 rhs buffers), so the
    PE never starves and the rings run back-to-back at ~320 GB/s each.
  - each chunk's output is one [128, 2, 1024] fp16 tile written back as
    a single 0.5 MB DMA on the ring opposite its read, so read and
    write streams interleave without blocking each other.
  - PSUM->SBUF copies are one 1024-wide (2-bank) copy per (chunk, mb):
    DVE takes mb=0, ACT mb=1 — ~1.4 us/engine/chunk, under the PE's
    1.73 us/chunk.
  - matmuls ordered so consecutive pairs share the same lhsT block,
    halving LDWEIGHTS pressure.
"""

import numpy as np
import ml_dtypes

B, C, H, W = 4, 256, 128, 256
N_CORES = 8
SPLIT_H = 2            # 8 shards = B(4) x H-halves(2)
SH = H // SPLIT_H      # 64 rows of H per shard
P_SHARD = SH * W       # 16384 pixels per core
TILE_N = 512
CH = 1024              # columns per chunk
N_CH = P_SHARD // CH   # 16

_NC_CACHE = {}


def _build_nc():
    if "nc" in _NC_CACHE:
        return _NC_CACHE["nc"]

    import concourse.bass as bass
    import concourse.tile as tile
    from concourse import bacc, mybir

    f32 = mybir.dt.float32
    f16 = mybir.dt.float16
    bf16 = mybir.dt.bfloat16

    nc = bacc.Bacc()
    feat = nc.dram_tensor("feat", [C, P_SHARD], bf16, kind="ExternalInput")
    mmat = nc.dram_tensor("mmat", [C, C], bf16, kind="ExternalInput")
    # out[p, mb, n] = recon[mb*128 + p, n]; packing both row-halves lets
    # one DMA per chunk carry 0.5 MB. fp16 halves write traffic; host
    # upcasts to fp32 (exact).
    out = nc.dram_tensor("out", [128, 2, P_SHARD], f16, kind="ExternalOutput")

    # feat rows are (kb*128 + p); view as [p, kb, n] so one DMA per chunk
    # pulls both K-halves.
    feat3 = feat.rearrange("(a k) n -> k a n", a=2)

    rd_eng = lambda c: nc.sync if (c % 2 == 0) else nc.scalar
    wr_eng = lambda c: nc.scalar if (c % 2 == 0) else nc.sync

    with tile.TileContext(nc) as tc:
        with (
            tc.tile_pool(name="mpool", bufs=1) as mpool,
            tc.tile_pool(name="rhs", bufs=N_CH) as rhs_pool,
            tc.tile_pool(name="opool", bufs=4) as opool,
            tc.tile_pool(name="psum", bufs=2, space="PSUM") as psum_pool,
        ):
            # M as two [128, 256] K-halves; lhsT block for (kb, mb) is
            # m_tiles[kb][:, mb*128:(mb+1)*128] (M is symmetric so lhsT = M).
            m_tiles = []
            for kb in range(2):
                mt = mpool.tile([128, C], bf16, tag=f"m{kb}")
                nc.scalar.dma_start(mt[:], mmat[kb * 128:(kb + 1) * 128, :])
                m_tiles.append(mt)

            # All reads issued up front, alternating rings, so each ring
            # streams its 8 chunks back-to-back.
            rts = []
            for c in range(N_CH):
                rt = rhs_pool.tile([128, 2, CH], bf16, tag="r", name=f"rt{c}")
                rd_eng(c).dma_start(rt[:], feat3[:, :, bass.ts(c, CH)])
                rts.append(rt)

            for c in range(N_CH):
                rt = rts[c]
                ot = opool.tile([128, 2, CH], f16, tag="o", name="ot")
                for mb in range(2):
                    ps = psum_pool.tile([128, CH], f32, tag=f"ps{mb}")
                    for kb in range(2):
                        for h in range(2):
                            nc.tensor.matmul(
                                ps[:, bass.ts(h, TILE_N)],
                                m_tiles[kb][:, mb * 128:(mb + 1) * 128],
                                rt[:, kb, bass.ts(h, TILE_N)],
                                start=(kb == 0),
                                stop=(kb == 1),
                            )
                    if mb == 0:
                        nc.vector.tensor_copy(ot[:, 0, :], ps[:])
                    else:
                        nc.scalar.copy(ot[:, 1, :], ps[:])
                wr_eng(c).dma_start(out[:, :, bass.ts(c, CH)], ot[:])

    nc.compile()
    _NC_CACHE["nc"] = nc
    return nc


def _host_prep(feature, codebook):
    cb = codebook.astype(np.float64)
    norm = np.sum(cb * cb, axis=1)
    m = ((cb / norm[:, None]).T @ cb).astype(ml_dtypes.bfloat16)

    feature = np.asarray(feature)
    in_maps = []
    for i in range(N_CORES):
        b, hs = i // SPLIT_H, (i % SPLIT_H) * SH
        shard = np.ascontiguousarray(
            feature[b, :, hs:hs + SH, :].reshape(C, P_SHARD)
        ).astype(ml_dtypes.bfloat16)
        in_maps.append({"feat": shard, "mmat": m})
    return in_maps


def _gather(results):
    out = np.empty((B, C, H, W), dtype=np.float32)
    for i in range(N_CORES):
        b, hs = i // SPLIT_H, (i % SPLIT_H) * SH
        r = results[i]["out"]  # [128, 2, P_SHARD] fp16
        out[b, :, hs:hs + SH, :] = (
            r.transpose(1, 0, 2).reshape(C, SH, W).astype(np.float32)
        )
    return out


def run(feature, codebook, **spmd_kwargs):
    from concourse.bass_utils import run_bass_kernel_spmd

    nc = _build_nc()
    in_maps = _host_prep(np.asarray(feature), np.asarray(codebook))
    res = run_bass_kernel_spmd(nc, in_maps, list(range(N_CORES)), **spmd_kwargs)
    return _gather(res.results), res


def kernel(feature, codebook):
    out, _ = run(feature, codebook)
    return out


# revision 6
# speedup vs baseline: 1.0862x; 1.0862x over previous
"""VQ codebook reconstruction kernel for Trainium2 (8 NeuronCores, SPMD).

Reference computation (per pixel feature vector f in R^C):
    weights = (codebook @ f) / ||codebook_rows||^2      # [N]
    recon   = codebook.T @ weights                      # [C]

This collapses to a single fixed matrix applied per pixel:
    recon = M @ f,   M = codebook.T @ diag(1/||c_n||^2) @ codebook   # [C, C]

M is tiny ([256,256]) and is formed on the host in float64; the device
kernel applies M to all B*H*W = 131072 pixel vectors, sharded
data-parallel over (B, H) across 8 cores.

v4 schedule (vs 65 us v2/v3, 81 us v1):
  - feature and M in bf16 (host cast, ~2^-9 rel err), output fp16.
  - reads at 2048-col granularity (4 KB/partition descriptors keep the
    HWDGE rings at ~315 GB/s; 1024-col DMAs halve that), alternating
    both rings, all issued up front into resident tiles. The first two
    chunks are 1024-col so the PE starts ~2 us earlier.
  - ~10 dummy matmuls on garbage SBUF warm the PE's HAM clock gate
    during the initial DMA wait, so real matmuls run at 2.4 GHz from
    the first chunk.
  - compute in 1024-col chunks: 8 matmuls (pairs share lhsT), one
    1024-wide 2-bank PSUM->SBUF copy per (chunk, mb). Copies split
    5:3 between DVE (1.31 us meas.) and ACT (1.52 us meas.) so
    neither exceeds the PE's 3.46 us per chunk pair.
  - writes are 1 MB per chunk pair: the first two on the otherwise
    idle SWDGE (gpsimd) ring, the rest alternating the two HWDGE
    rings once their reads have drained.
"""

import numpy as np
import ml_dtypes

B, C, H, W = 4, 256, 128, 256
N_CORES = 8
SPLIT_H = 2            # 8 shards = B(4) x H-halves(2)
SH = H // SPLIT_H      # 64 rows of H per shard
P_SHARD = SH * W       # 16384 pixels per core
TILE_N = 512
CH = 1024              # compute-chunk columns
N_CH = P_SHARD // CH   # 16
N_WARM = 10            # PE warmup matmuls

_NC_CACHE = {}


def _build_nc():
    if "nc" in _NC_CACHE:
        return _NC_CACHE["nc"]

    import concourse.bass as bass
    import concourse.tile as tile
    from concourse import bacc, mybir

    f32 = mybir.dt.float32
    f16 = mybir.dt.float16
    bf16 = mybir.dt.bfloat16

    nc = bacc.Bacc()
    feat = nc.dram_tensor("feat", [C, P_SHARD], bf16, kind="ExternalInput")
    mmat = nc.dram_tensor("mmat", [C, C], bf16, kind="ExternalInput")
    # out[p, mb, n] = recon[mb*128 + p, n]; packing both row-halves lets
    # one DMA per chunk pair carry 1 MB. fp16 halves write traffic; host
    # upcasts to fp32 (exact).
    out = nc.dram_tensor("out", [128, 2, P_SHARD], f16, kind="ExternalOutput")

    # feat rows are (kb*128 + p); view as [p, kb, n] so one DMA per chunk
    # pulls both K-halves.
    feat3 = feat.rearrange("(a k) n -> k a n", a=2)

    with tile.TileContext(nc) as tc:
        with (
            tc.tile_pool(name="mpool", bufs=1) as mpool,
            tc.tile_pool(name="rhs", bufs=1) as rhs_pool,
            tc.tile_pool(name="warm", bufs=1) as warm_pool,
            tc.tile_pool(name="opool", bufs=3) as opool,
            tc.tile_pool(name="psum", bufs=2, space="PSUM") as psum_pool,
        ):
            # M as two [128, 256] K-halves; lhsT block for (kb, mb) is
            # m_tiles[kb][:, mb*128:(mb+1)*128] (M is symmetric so lhsT = M).
            m_tiles = []
            for kb in range(2):
                mt = mpool.tile([128, C], bf16, tag=f"m{kb}")
                nc.scalar.dma_start(mt[:], mmat[kb * 128:(kb + 1) * 128, :])
                m_tiles.append(mt)

            # Reads, all issued up front into resident tiles.
            # ring A (sync): cols 0-1023, then (2,3), (6,7), (10,11), (14,15)
            # ring B (scalar): M, cols 1024-2047, then (4,5), (8,9), (12,13)
            reads = [
                (nc.sync, 0, 1),      # (engine, first chunk, n chunks)
                (nc.scalar, 1, 1),
                (nc.sync, 2, 2),
                (nc.scalar, 4, 2),
                (nc.sync, 6, 2),
                (nc.scalar, 8, 2),
                (nc.sync, 10, 2),
                (nc.scalar, 12, 2),
                (nc.sync, 14, 2),
            ]
            chunk_src = {}
            for eng, c0, nch in reads:
                rt = rhs_pool.tile([128, 2, nch * CH], bf16, tag=f"rt{c0}",
                                   name=f"rt{c0}")
                eng.dma_start(rt[:], feat3[:, :, bass.ds(c0 * CH, nch * CH)])
                for c in range(c0, c0 + nch):
                    chunk_src[c] = (rt, (c - c0) * CH)

            # PE warmup on garbage SBUF into the first ps0 tile's bank;
            # start=True on every op so nothing accumulates, and chunk 0's
            # real matmuls overwrite (start=True clears the bank). This
            # keeps the PE busy through the HAM activity window during the
            # first DMA waits so real matmuls run warm (2.4 GHz).
            wt = warm_pool.tile([128, TILE_N], bf16, tag="w")
            nc.gpsimd.memset(wt[:], 0.0)
            warm_ps = []
            for mb in range(2):
                ps = psum_pool.tile([128, CH], f32, tag=f"ps{mb}")
                warm_ps.append(ps)
                for i in range(N_WARM // 2):
                    nc.tensor.matmul(
                        ps[:, 0:TILE_N], wt[:, 0:128], wt[:],
                        start=True, stop=True, skip_group_check=True,
                    )

            # Copy engine per (chunk, mb): 5:3 DVE:ACT over each 4-chunk
            # period (DVE 1.31 us/copy vs ACT 1.52 us measured).
            act_copy = {(1, 1), (2, 0), (3, 1)}  # (c % 4, mb) -> ACT

            # Writes: 1 MB per chunk pair; first two pairs on SWDGE
            # (gpsimd), then alternating HWDGE rings as reads drain.
            wr_eng = [nc.gpsimd, nc.gpsimd, nc.scalar, nc.sync,
                      nc.scalar, nc.sync, nc.scalar, nc.sync]

            ot = None
            for c in range(N_CH):
                rt, off = chunk_src[c]
                if c % 2 == 0:
                    ot = opool.tile([128, 2, 2 * CH], f16, tag="o", name="ot")
                for mb in range(2):
                    if c == 0:
                        ps = warm_ps[mb]
                    else:
                        ps = psum_pool.tile([128, CH], f32, tag=f"ps{mb}")
                    for kb in range(2):
                        for h in range(2):
                            nc.tensor.matmul(
                                ps[:, bass.ts(h, TILE_N)],
                                m_tiles[kb][:, mb * 128:(mb + 1) * 128],
                                rt[:, kb, bass.ds(off + h * TILE_N, TILE_N)],
                                start=(kb == 0),
                                stop=(kb == 1),
                                skip_group_check=(c == 0),
                            )
                    dest = ot[:, mb, bass.ts(c % 2, CH)]
                    if (c % 4, mb) in act_copy:
                        nc.scalar.copy(dest, ps[:])
                    else:
                        nc.vector.tensor_copy(dest, ps[:])
                if c % 2 == 1:
                    wr_eng[c // 2].dma_start(
                        out[:, :, bass.ds((c - 1) * CH, 2 * CH)], ot[:]
                    )

    nc.compile()
    _NC_CACHE["nc"] = nc
    return nc


def _host_prep(feature, codebook):
    cb = codebook.astype(np.float64)
    norm = np.sum(cb * cb, axis=1)
    m = ((cb / norm[:, None]).T @ cb).astype(ml_dtypes.bfloat16)

    feature = np.asarray(feature)
    in_maps = []
    for i in range(N_CORES):
        b, hs = i // SPLIT_H, (i % SPLIT_H) * SH
        shard = np.ascontiguousarray(
            feature[b, :, hs:hs + SH, :].reshape(C, P_SHARD)
        ).astype(ml_dtypes.bfloat16)
        in_maps.append({"feat": shard, "mmat": m})
    return in_maps


def _gather(results):
    out = np.empty((B, C, H, W), dtype=np.float32)
    for i in range(N_CORES):
        b, hs = i // SPLIT_H, (i % SPLIT_H) * SH
        r = results[i]["out"]  # [128, 2, P_SHARD] fp16
        out[b, :, hs:hs + SH, :] = (
            r.transpose(1, 0, 2).reshape(C, SH, W).astype(np.float32)
        )
    return out


def run(feature, codebook, **spmd_kwargs):
    from concourse.bass_utils import run_bass_kernel_spmd

    nc = _build_nc()
    in_maps = _host_prep(np.asarray(feature), np.asarray(codebook))
    res = run_bass_kernel_spmd(nc, in_maps, list(range(N_CORES)), **spmd_kwargs)
    return _gather(res.results), res


def kernel(feature, codebook):
    out, _ = run(feature, codebook)
    return out


# revision 7
# speedup vs baseline: 1.1812x; 1.0875x over previous
"""VQ codebook reconstruction kernel for Trainium2 (8 NeuronCores, SPMD).

Reference computation (per pixel feature vector f in R^C):
    weights = (codebook @ f) / ||codebook_rows||^2      # [N]
    recon   = codebook.T @ weights                      # [C]

This collapses to a single fixed matrix applied per pixel:
    recon = M @ f,   M = codebook.T @ diag(1/||c_n||^2) @ codebook   # [C, C]

M is tiny ([256,256]) and is formed on the host in float64; the device
kernel applies M to all B*H*W = 131072 pixel vectors, sharded
data-parallel over (B, H) across 8 cores.

The kernel is DMA-byte-bound: 8.39 MB bf16 in + 8.39 MB fp16 out per
core against an ~420 GB/s aggregate SDMA/fabric ceiling (~40 us).
fp8 inputs would halve reads but fail the 2e-2 gate (measured 3.9e-2).
So v5 packs the DMA timeline:
  - 16 reads of 0.5 MB alternate the two HWDGE rings. Even chunks are
    issued up front on sync (blocking there is harmless); odd chunks
    are issued from the compute loop with 6-chunk lookahead so the
    scalar engine's triggers never block its ACT copies (ring credit
    is ~4 outstanding DMAs; a blocked trigger stalls every copy queued
    behind it — that was v4's mid-kernel stall).
  - writes: first three 1 MB pairs on SWDGE (gpsimd), later pairs
    alternate the HWDGE rings as their reads drain, final pair split
    0.5 MB + 0.5 MB across both rings to shorten the tail.
  - 8 warmup matmuls on zeroed SBUF bring the PE out of the HAM
    cold state (1.2 -> 2.4 GHz) before the first real chunk lands.
  - compute in 1024-col chunks: 8 matmuls (pairs share lhsT), one
    1024-wide 2-bank PSUM->SBUF copy per (chunk, mb), split 5:3
    between DVE (1.31 us meas.) and ACT (1.52 us meas.).
"""

import numpy as np
import ml_dtypes

B, C, H, W = 4, 256, 128, 256
N_CORES = 8
SPLIT_H = 2            # 8 shards = B(4) x H-halves(2)
SH = H // SPLIT_H      # 64 rows of H per shard
P_SHARD = SH * W       # 16384 pixels per core
TILE_N = 512
CH = 1024              # compute-chunk columns
N_CH = P_SHARD // CH   # 16
N_WARM = 8             # PE warmup matmuls
LOOKAHEAD = 6          # chunks of read prefetch on the scalar ring

_NC_CACHE = {}


def _build_nc():
    if "nc" in _NC_CACHE:
        return _NC_CACHE["nc"]

    import concourse.bass as bass
    import concourse.tile as tile
    from concourse import bacc, mybir

    f32 = mybir.dt.float32
    f16 = mybir.dt.float16
    bf16 = mybir.dt.bfloat16

    nc = bacc.Bacc()
    feat = nc.dram_tensor("feat", [C, P_SHARD], bf16, kind="ExternalInput")
    mmat = nc.dram_tensor("mmat", [C, C], bf16, kind="ExternalInput")
    # out[p, mb, n] = recon[mb*128 + p, n]; fp16 halves write traffic;
    # host upcasts to fp32 (exact).
    out = nc.dram_tensor("out", [128, 2, P_SHARD], f16, kind="ExternalOutput")

    # feat rows are (kb*128 + p); view as [p, kb, n] so one DMA per chunk
    # pulls both K-halves.
    feat3 = feat.rearrange("(a k) n -> k a n", a=2)

    with tile.TileContext(nc) as tc:
        with (
            tc.tile_pool(name="mpool", bufs=1) as mpool,
            tc.tile_pool(name="rhs", bufs=1) as rhs_pool,
            tc.tile_pool(name="warm", bufs=1) as warm_pool,
            tc.tile_pool(name="opool", bufs=3) as opool,
            tc.tile_pool(name="psum", bufs=2, space="PSUM") as psum_pool,
        ):
            # M as two [128, 256] K-halves; lhsT block for (kb, mb) is
            # m_tiles[kb][:, mb*128:(mb+1)*128] (M is symmetric so lhsT = M).
            # M rides the scalar ring first, ahead of that ring's reads.
            m_tiles = []
            for kb in range(2):
                mt = mpool.tile([128, C], bf16, tag=f"m{kb}")
                nc.scalar.dma_start(mt[:], mmat[kb * 128:(kb + 1) * 128, :])
                m_tiles.append(mt)

            rhs_tiles = {}

            def read_chunk(c, eng):
                rt = rhs_pool.tile([128, 2, CH], bf16, tag=f"rt{c}", name=f"rt{c}")
                eng.dma_start(rt[:], feat3[:, :, bass.ts(c, CH)])
                rhs_tiles[c] = rt

            # Even chunks: all up front on the sync ring.
            for c in range(0, N_CH, 2):
                read_chunk(c, nc.sync)
            # First few odd chunks up front on scalar (stays under the
            # ring-credit limit); the rest are issued from the compute loop.
            for c in range(1, LOOKAHEAD, 2):
                read_chunk(c, nc.scalar)

            # PE warmup on zeroed SBUF into the first ps tiles' banks;
            # chunk 0's real matmuls overwrite them (start=True clears).
            # Keeps the PE busy through the HAM activity window so real
            # matmuls run at 2.4 GHz from the start.
            wt = warm_pool.tile([128, TILE_N], bf16, tag="w")
            nc.gpsimd.memset(wt[:], 0.0)
            warm_ps = []
            for mb in range(2):
                ps = psum_pool.tile([128, CH], f32, tag=f"ps{mb}")
                warm_ps.append(ps)
                for i in range(N_WARM // 2):
                    nc.tensor.matmul(
                        ps[:, 0:TILE_N], wt[:, 0:128], wt[:],
                        start=True, stop=True, skip_group_check=True,
                    )

            # Copy engine per (chunk, mb): 5:3 DVE:ACT over each 4-chunk
            # period (DVE 1.31 us/copy vs ACT 1.52 us measured).
            act_copy = {(1, 1), (2, 0), (3, 1)}  # (c % 4, mb) -> ACT

            # Writes per chunk pair: SWDGE first, HWDGE rings once their
            # reads drain; None = final pair, split across both rings.
            wr_eng = [nc.gpsimd, nc.gpsimd, nc.gpsimd, nc.scalar,
                      nc.sync, nc.scalar, nc.sync, None]

            ot = None
            for c in range(N_CH):
                rt = rhs_tiles[c]
                if c % 2 == 0:
                    ot = opool.tile([128, 2, 2 * CH], f16, tag="o", name="ot")
                for mb in range(2):
                    if c == 0:
                        ps = warm_ps[mb]
                    else:
                        ps = psum_pool.tile([128, CH], f32, tag=f"ps{mb}")
                    for kb in range(2):
                        for h in range(2):
                            nc.tensor.matmul(
                                ps[:, bass.ts(h, TILE_N)],
                                m_tiles[kb][:, mb * 128:(mb + 1) * 128],
                                rt[:, kb, bass.ts(h, TILE_N)],
                                start=(kb == 0),
                                stop=(kb == 1),
                                skip_group_check=(c == 0),
                            )
                    dest = ot[:, mb, bass.ts(c % 2, CH)]
                    if (c % 4, mb) in act_copy:
                        nc.scalar.copy(dest, ps[:])
                    else:
                        nc.vector.tensor_copy(dest, ps[:])
                # Prefetch the odd chunk LOOKAHEAD ahead on the scalar
                # ring, after this chunk's copies so the trigger sits
                # behind at most ~3 in-flight DMAs (never blocks ACT).
                tgt = c + LOOKAHEAD
                if tgt < N_CH and tgt % 2 == 1:
                    read_chunk(tgt, nc.scalar)
                if c % 2 == 1:
                    pair = c // 2
                    dst = out[:, :, bass.ds((c - 1) * CH, 2 * CH)]
                    if wr_eng[pair] is not None:
                        wr_eng[pair].dma_start(dst, ot[:])
                    else:
                        nc.scalar.dma_start(
                            out[:, :, bass.ds((c - 1) * CH, CH)],
                            ot[:, :, 0:CH],
                        )
                        nc.sync.dma_start(
                            out[:, :, bass.ds(c * CH, CH)],
                            ot[:, :, CH:2 * CH],
                        )

    nc.compile()
    _NC_CACHE["nc"] = nc
    return nc


def _host_prep(feature, codebook):
    cb = codebook.astype(np.float64)
    norm = np.sum(cb * cb, axis=1)
    m = ((cb / norm[:, None]).T @ cb).astype(ml_dtypes.bfloat16)

    feature = np.asarray(feature)
    in_maps = []
    for i in range(N_CORES):
        b, hs = i // SPLIT_H, (i % SPLIT_H) * SH
        shard = np.ascontiguousarray(
            feature[b, :, hs:hs + SH, :].reshape(C, P_SHARD)
        ).astype(ml_dtypes.bfloat16)
        in_maps.append({"feat": shard, "mmat": m})
    return in_maps


def _gather(results):
    out = np.empty((B, C, H, W), dtype=np.float32)
    for i in range(N_CORES):
        b, hs = i // SPLIT_H, (i % SPLIT_H) * SH
        r = results[i]["out"]  # [128, 2, P_SHARD] fp16
        out[b, :, hs:hs + SH, :] = (
            r.transpose(1, 0, 2).reshape(C, SH, W).astype(np.float32)
        )
    return out


def run(feature, codebook, **spmd_kwargs):
    from concourse.bass_utils import run_bass_kernel_spmd

    nc = _build_nc()
    in_maps = _host_prep(np.asarray(feature), np.asarray(codebook))
    res = run_bass_kernel_spmd(nc, in_maps, list(range(N_CORES)), **spmd_kwargs)
    return _gather(res.results), res


def kernel(feature, codebook):
    out, _ = run(feature, codebook)
    return out
